# revision 8
# baseline (speedup 1.0000x reference)
"""HAN layer (3-metapath GraphConv + semantic attention) on 8 Trainium2 cores.

Strategy (per sharding hint): partition nodes by dst across the 8 cores; each
core owns the edges targeting its 6250 dst nodes.  Edges are sorted by dst on
the host so the scatter-add becomes per-128-dst-block one-hot matmuls on the
TensorEngine:

    agg[f, u] += G[e, f].T @ S'[e, u]
      G  = dma_gather of h rows (fp16, 256B rows) for the tile's 128 src ids
      S' = (iota[u] == dst_local[e]) * coef[e]   built by one DVE tensor_scalar
      coef = 1/sqrt(deg_out[src]) * 1/sqrt(deg_in[dst])  (graph-structure
             normalization, precomputed on host like the edge partitioning)

    z[dout, u] = W_p.T @ agg  + b_p       (per block; W_p stationary)

Semantic attention: per block, psumA[u,h] = b1 + z.T @ w1; tanh on ACT;
DVE mult by broadcast w2 and row-reduce; per-path partials are summed with a
ones-matmul, AllReduced across the 8 cores, softmaxed on-device, and the
final out[dout, u] = sum_p beta_p * z_p is DMA'd out (host transposes back).

dma_gather indices are int16, so the h table is split at row 32768 into lo/hi
halves and each core's edge stream is kept as separate lo/hi substreams.
"""

import sys

sys.path.insert(0, "/opt/trn_rl_repo")

import numpy as np

import concourse.bacc as bacc
import concourse.mybir as mybir
import concourse.tile as tile
from concourse import bass_utils

N_NODES = 50000
N_EDGES = 600000
NPATH = 3
D = 128
N_CORES = 8
NPC = N_NODES // N_CORES          # 6250 dst nodes per core
NBLK = (NPC + 127) // 128         # 49 dst blocks per core (last has 106 rows)
LAST_ROWS = NPC - (NBLK - 1) * 128
SPLIT = 32768                     # int16 gather index limit
CHUNK = 2048                      # edges per dma_gather call
USE_GATHER = True                 # debug: False -> plain DMA instead

f16 = mybir.dt.float16
f32 = mybir.dt.float32
i16 = mybir.dt.int16


def _pack_idx(idx_flat):
    """int16 edge ids -> [128, n/16] layout: j -> [j%16, j//16], tiled x8."""
    n = len(idx_flat)
    assert n % 16 == 0
    a = idx_flat.reshape(n // 16, 16).T
    return np.tile(a, (8, 1)).copy()


def _pack_cols(v_flat, n_tiles):
    """per-edge value -> [128, n_tiles]: edge (t*128+p) at [p, t]."""
    return v_flat.reshape(n_tiles, 128).T.copy()


def _prep(h, W_gc, b_gc, w1, b1, w2, edge_src, edge_dst):
    """Host-side sharding: per-core dst-sorted padded edge streams + budgets."""
    s_out = np.zeros((NPATH, N_NODES), np.float32)
    s_in = np.zeros((NPATH, N_NODES), np.float32)
    for p in range(NPATH):
        do = np.bincount(edge_src[p], minlength=N_NODES).astype(np.float32)
        di = np.bincount(edge_dst[p], minlength=N_NODES).astype(np.float32)
        s_out[p] = 1.0 / np.sqrt(np.maximum(do, 1.0))
        s_in[p] = 1.0 / np.sqrt(np.maximum(di, 1.0))

    # per (core, path, block): lo/hi edge lists sorted by dst
    segs = {}
    for p in range(NPATH):
        src = edge_src[p].astype(np.int64)
        dst = edge_dst[p].astype(np.int64)
        core = dst // NPC
        for c in range(N_CORES):
            m = core == c
            s_c, d_c = src[m], dst[m]
            order = np.argsort(d_c, kind="stable")
            s_c, d_c = s_c[order], d_c[order]
            blk = (d_c - c * NPC) // 128
            bounds = np.searchsorted(blk, np.arange(NBLK + 1))
            for b in range(NBLK):
                lo, hi_ = bounds[b], bounds[b + 1]
                s_b, d_b = s_c[lo:hi_], d_c[lo:hi_]
                is_hi = s_b >= SPLIT
                segs[c, p, b] = (s_b[~is_hi], d_b[~is_hi], s_b[is_hi], d_b[is_hi])

    # SPMD budgets: tiles per (path, block, half), max over cores
    bud_lo = np.zeros((NPATH, NBLK), np.int64)
    bud_hi = np.zeros((NPATH, NBLK), np.int64)
    for (c, p, b), (sl, _, sh, _) in segs.items():
        bud_lo[p, b] = max(bud_lo[p, b], -(-len(sl) // 128))
        bud_hi[p, b] = max(bud_hi[p, b], -(-len(sh) // 128))
    bud_lo = np.maximum(bud_lo, 1)
    bud_hi = np.maximum(bud_hi, 1)

    n_tiles = int((bud_lo + bud_hi).sum())
    lo_edges = int(bud_lo.sum()) * 128
    hi_edges = int(bud_hi.sum()) * 128
    lo_pad = -lo_edges % CHUNK
    hi_pad = -hi_edges % CHUNK

    in_maps = []
    t_lo = np.ascontiguousarray(h[:SPLIT]).astype(np.float16)
    t_hi = np.ascontiguousarray(h[SPLIT:]).astype(np.float16)
    w1f = w1.astype(np.float16)
    b1row = b1.reshape(1, D).astype(np.float16)
    w2b = np.tile(w2.reshape(1, D), (128, 1)).astype(np.float16)
    wgc = W_gc.astype(np.float16)              # [p][f_in, d_out]
    bgc = np.ascontiguousarray(b_gc.T).astype(np.float32)   # [128, 3]
    iota = np.tile(np.arange(128, dtype=np.float16)[None, :], (128, 1))
    maskcol = (np.arange(128) < LAST_ROWS).astype(np.float32).reshape(128, 1)
    ones128 = np.ones((128, 1), np.float32)
    one1x128f16 = np.ones((1, 128), np.float16)

    for c in range(N_CORES):
        il, ih, dl_, cf = [], [], [], []
        for p in range(NPATH):
            for b in range(NBLK):
                sl, dlo, sh, dhi = segs[c, p, b]
                base = c * NPC + b * 128
                for (s_b, d_b, bud, off) in (
                    (sl, dlo, bud_lo[p, b], 0),
                    (sh, dhi, bud_hi[p, b], SPLIT),
                ):
                    npad = int(bud) * 128 - len(s_b)
                    idx = np.concatenate([s_b - off, np.zeros(npad, np.int64)])
                    dst_l = np.concatenate([d_b - base, np.zeros(npad, np.int64)])
                    coef = np.concatenate(
                        [s_out[p, s_b] * s_in[p, d_b], np.zeros(npad, np.float32)]
                    )
                    (il if off == 0 else ih).append(idx)
                    dl_.append(dst_l)
                    cf.append(coef)
        idx_lo = np.concatenate(il + [np.zeros(lo_pad, np.int64)]).astype(np.int16)
        idx_hi = np.concatenate(ih + [np.zeros(hi_pad, np.int64)]).astype(np.int16)
        dstl = np.concatenate(dl_).astype(np.float32)
        coef = np.concatenate(cf).astype(np.float32)
        in_maps.append(
            {
                "t_lo": t_lo,
                "t_hi": t_hi,
                "idx_lo": _pack_idx(idx_lo),
                "idx_hi": _pack_idx(idx_hi),
                "dstl": _pack_cols(dstl, n_tiles),
                "coef": _pack_cols(coef, n_tiles),
                "w1f": w1f,
                "b1row": b1row,
                "w2b": w2b,
                "wgc0": wgc[0],
                "wgc1": wgc[1],
                "wgc2": wgc[2],
                "bgc": bgc,
                "iota": iota,
                "maskcol": maskcol,
                "ones128": ones128,
                "one1x128f16": one1x128f16,
            }
        )
    return in_maps, bud_lo, bud_hi, n_tiles, lo_edges + lo_pad, hi_edges + hi_pad


def _build(bud_lo, bud_hi, n_tiles, lo_total, hi_total, stage=2, limit=None):
    nc = bacc.Bacc("TRN2", target_bir_lowering=False, debug=False,
                   num_devices=N_CORES)

    t_lo = nc.dram_tensor("t_lo", [SPLIT, D], f16, kind="ExternalInput")
    t_hi = nc.dram_tensor("t_hi", [N_NODES - SPLIT, D], f16, kind="ExternalInput")
    idx_lo = nc.dram_tensor("idx_lo", [128, lo_total // 16], i16, kind="ExternalInput")
    idx_hi = nc.dram_tensor("idx_hi", [128, hi_total // 16], i16, kind="ExternalInput")
    dstl = nc.dram_tensor("dstl", [128, n_tiles], f32, kind="ExternalInput")
    coef = nc.dram_tensor("coef", [128, n_tiles], f32, kind="ExternalInput")
    w1f = nc.dram_tensor("w1f", [D, D], f16, kind="ExternalInput")
    b1row = nc.dram_tensor("b1row", [1, D], f16, kind="ExternalInput")
    w2b = nc.dram_tensor("w2b", [128, D], f16, kind="ExternalInput")
    wgc = [nc.dram_tensor(f"wgc{p}", [D, D], f16, kind="ExternalInput")
           for p in range(NPATH)]
    bgc = nc.dram_tensor("bgc", [128, NPATH], f32, kind="ExternalInput")
    iota_in = nc.dram_tensor("iota", [128, 128], f16, kind="ExternalInput")
    maskcol = nc.dram_tensor("maskcol", [128, 1], f32, kind="ExternalInput")
    ones128 = nc.dram_tensor("ones128", [128, 1], f32, kind="ExternalInput")
    one1x128f16 = nc.dram_tensor("one1x128f16", [1, 128], f16, kind="ExternalInput")
    out = nc.dram_tensor("out", [128, NBLK * 128], f32, kind="ExternalOutput")
    z_out = (nc.dram_tensor("z_out", [128, NPATH * NBLK * 128], f16,
                            kind="ExternalOutput") if stage == 1 else None)

    cci = nc.dram_tensor("cci", [1, NPATH], f32, kind="Internal")
    cco = nc.dram_tensor("cco", [1, NPATH], f32, kind="Internal",
                         addr_space="Shared")

    with tile.TileContext(nc) as tc:
        with (
            tc.tile_pool(name="persist", bufs=1) as pp,
            tc.tile_pool(name="chunks", bufs=4) as cp,
            tc.tile_pool(name="work", bufs=4) as wp,
            tc.tile_pool(name="psum_main", bufs=2, space="PSUM") as pm,
            tc.tile_pool(name="psum_aux", bufs=2, space="PSUM") as pa,
        ):
            # --- persistent loads -------------------------------------------
            def load(dram, shape, dt, tag):
                t = pp.tile(shape, dt, tag=tag)
                nc.sync.dma_start(t[:], dram[:])
                return t

            idx_lo_t = load(idx_lo, [128, lo_total // 16], i16, "idx_lo")
            idx_hi_t = load(idx_hi, [128, hi_total // 16], i16, "idx_hi")
            dstl_t = load(dstl, [128, n_tiles], f32, "dstl")
            coef_t = load(coef, [128, n_tiles], f32, "coef")
            w1_t = load(w1f, [D, D], f16, "w1")
            b1_t = load(b1row, [1, D], f16, "b1")
            w2b_t = load(w2b, [128, D], f16, "w2b")
            wgc_t = [load(wgc[p], [D, D], f16, f"wgc{p}") for p in range(NPATH)]
            bgc_t = load(bgc, [128, NPATH], f32, "bgc")
            iota_t = load(iota_in, [128, 128], f16, "iota")
            mask_t = load(maskcol, [128, 1], f32, "mask")
            ones_t = load(ones128, [128, 1], f32, "ones")
            one1_t = load(one1x128f16, [1, 128], f16, "one1")

            z_all = pp.tile([128, NPATH * NBLK * 128], f16)     # [dout, u]
            out_sb = pp.tile([128, NBLK * 128], f32)

            # --- streaming gather state -------------------------------------
            state = {"lo": [0, None], "hi": [0, None]}
            tbl = {"lo": t_lo, "hi": t_hi}
            idxt = {"lo": idx_lo_t, "hi": idx_hi_t}

            def next_tile(stream):
                pos, cur = state[stream]
                k, slot = divmod(pos, CHUNK // 128)
                if slot == 0:
                    cur = cp.tile([128, CHUNK // 128, D], f16, tag=stream)
                    if USE_GATHER:
                        nc.gpsimd.dma_gather(
                            cur[:], tbl[stream][:],
                            idxt[stream][:, k * (CHUNK // 16):(k + 1) * (CHUNK // 16)],
                            CHUNK, CHUNK, D, single_packet=False)
                    else:
                        for jj in range(CHUNK // 128):
                            nc.sync.dma_start(cur[:, jj, :],
                                              tbl[stream][jj * 128:(jj + 1) * 128, :])
                    state[stream][1] = cur
                state[stream][0] = pos + 1
                return cur[:, slot, :]

            # --- main pass: per (path, block) aggregation + W matmul --------
            tpos = 0
            acc3 = pp.tile([128, NPATH], f32)
            lim_p, lim_b = limit if limit else (NPATH, NBLK)
            for p in range(lim_p):
                for b in range(NBLK):
                    if b >= lim_b:
                        break
                    nt = int(bud_lo[p, b] + bud_hi[p, b])
                    psum = pm.tile([128, 128], f32, tag="agg")
                    for j in range(nt):
                        g = next_tile("lo" if j < bud_lo[p, b] else "hi")
                        s = wp.tile([128, 128], f16, tag="s")
                        nc.vector.tensor_scalar(
                            s[:], iota_t[:],
                            dstl_t[:, tpos:tpos + 1], coef_t[:, tpos:tpos + 1],
                            op0=mybir.AluOpType.is_equal,
                            op1=mybir.AluOpType.mult)
                        nc.tensor.matmul(psum[:], g, s[:],
                                         start=(j == 0), stop=(j == nt - 1))
                        tpos += 1
                    agg = wp.tile([128, 128], f16, tag="agg_sb")
                    nc.vector.tensor_copy(agg[:], psum[:])
                    psz = pm.tile([128, 128], f32, tag="z")
                    nc.tensor.matmul(psz[:], wgc_t[p][:], agg[:],
                                     start=True, stop=True)
                    zt = z_all[:, (p * NBLK + b) * 128:(p * NBLK + b + 1) * 128]
                    nc.vector.tensor_scalar(zt, psz[:], bgc_t[:, p:p + 1], None,
                                            op0=mybir.AluOpType.add)

                if stage == 1:
                    continue
                # --- attention partial for path p ---------------------------
                accp = pp.tile([128, 1], f32, tag=f"accp{p}")
                for b in range(NBLK):
                    zt = z_all[:, (p * NBLK + b) * 128:(p * NBLK + b + 1) * 128]
                    psa = pa.tile([128, D], f32, tag="attn")
                    nc.tensor.matmul(psa[:], one1_t[:], b1_t[:],
                                     start=True, stop=False)
                    nc.tensor.matmul(psa[:], zt, w1_t[:], start=False, stop=True)
                    t_ = wp.tile([128, D], f16, tag="tanh")
                    nc.scalar.activation(t_[:], psa[:],
                                         mybir.ActivationFunctionType.Tanh)
                    m_ = wp.tile([128, D], f16, tag="tw2")
                    nc.vector.tensor_tensor(m_[:], t_[:], w2b_t[:],
                                            op=mybir.AluOpType.mult)
                    r_ = wp.tile([128, 1], f32, tag="r")
                    nc.vector.tensor_reduce(r_[:], m_[:],
                                            op=mybir.AluOpType.add,
                                            axis=mybir.AxisListType.X)
                    if b == NBLK - 1:
                        nc.vector.tensor_tensor(r_[:], r_[:], mask_t[:],
                                                op=mybir.AluOpType.mult)
                    if b == 0:
                        nc.vector.tensor_copy(accp[:], r_[:])
                    else:
                        nc.vector.tensor_tensor(accp[:], accp[:], r_[:],
                                                op=mybir.AluOpType.add)
                nc.vector.tensor_copy(acc3[:, p:p + 1], accp[:])

            if stage == 1:
                nc.sync.dma_start(z_out[:], z_all[:])
            else:
                # --- scores -> AllReduce -> softmax -> beta -----------------
                pss = pa.tile([1, NPATH], f32, tag="attn")
                nc.tensor.matmul(pss[:], ones_t[:], acc3[:], start=True, stop=True)
                s3 = pp.tile([1, NPATH], f32)
                nc.vector.tensor_copy(s3[:], pss[:])
                nc.sync.dma_start(cci[:], s3[:])
                nc.gpsimd.collective_compute(
                    "AllReduce", mybir.AluOpType.add,
                    replica_groups=[list(range(N_CORES))],
                    ins=[cci[:]], outs=[cco[:]])
                sred = pp.tile([1, NPATH], f32)
                nc.sync.dma_start(sred[:], cco[:])
                e3 = pp.tile([1, NPATH], f32)
                nc.scalar.activation(e3[:], sred[:],
                                     mybir.ActivationFunctionType.Exp,
                                     scale=1.0 / N_NODES)
                esum = pp.tile([1, 1], f32)
                nc.vector.tensor_reduce(esum[:], e3[:], op=mybir.AluOpType.add,
                                        axis=mybir.AxisListType.X)
                erec = pp.tile([1, 1], f32)
                nc.vector.reciprocal(erec[:], esum[:])
                beta_row = pp.tile([1, NPATH], f32)
                nc.vector.tensor_scalar(beta_row[:], e3[:], erec[:], None,
                                        op0=mybir.AluOpType.mult)
                onef = pp.tile([1, 128], f32)
                nc.vector.tensor_copy(onef[:], one1_t[:])
                psb = pa.tile([128, NPATH], f32, tag="attn")
                nc.tensor.matmul(psb[:], onef[:], beta_row[:], start=True, stop=True)
                betab = pp.tile([128, NPATH], f32)
                nc.vector.tensor_copy(betab[:], psb[:])

                # --- combine ------------------------------------------------
                for b in range(NBLK):
                    o = out_sb[:, b * 128:(b + 1) * 128]
                    z0 = z_all[:, (0 * NBLK + b) * 128:(0 * NBLK + b + 1) * 128]
                    z1 = z_all[:, (1 * NBLK + b) * 128:(1 * NBLK + b + 1) * 128]
                    z2 = z_all[:, (2 * NBLK + b) * 128:(2 * NBLK + b + 1) * 128]
                    q = wp.tile([128, 128], f32, tag="q")
                    nc.vector.tensor_scalar(o, z0, betab[:, 0:1], None,
                                            op0=mybir.AluOpType.mult)
                    nc.vector.tensor_scalar(q[:], z1, betab[:, 1:2], None,
                                            op0=mybir.AluOpType.mult)
                    nc.vector.tensor_tensor(o, o, q[:], op=mybir.AluOpType.add)
                    nc.vector.tensor_scalar(q[:], z2, betab[:, 2:3], None,
                                            op0=mybir.AluOpType.mult)
                    nc.vector.tensor_tensor(o, o, q[:], op=mybir.AluOpType.add)
                nc.sync.dma_start(out[:], out_sb[:])

    nc.compile()
    return nc


_CACHE = {}


def kernel(**inputs):
    h = np.asarray(inputs["h"], np.float32)
    W_gc = np.asarray(inputs["W_gc"], np.float32)
    b_gc = np.asarray(inputs["b_gc"], np.float32)
    w1 = np.asarray(inputs["w1"], np.float32)
    b1 = np.asarray(inputs["b1"], np.float32)
    w2 = np.asarray(inputs["w2"], np.float32)
    edge_src = np.asarray(inputs["edge_src"])
    edge_dst = np.asarray(inputs["edge_dst"])

    in_maps, bud_lo, bud_hi, n_tiles, lo_total, hi_total = _prep(
        h, W_gc, b_gc, w1, b1, w2, edge_src, edge_dst)

    key = (bud_lo.tobytes(), bud_hi.tobytes())
    if key not in _CACHE:
        _CACHE[key] = _build(bud_lo, bud_hi, n_tiles, lo_total, hi_total)
    nc = _CACHE[key]

    res = bass_utils.run_bass_kernel_spmd(nc, in_maps, core_ids=list(range(N_CORES)))
    out = np.empty((N_NODES, D), np.float32)
    for c in range(N_CORES):
        out[c * NPC:(c + 1) * NPC] = res.results[c]["out"][:, :NPC].T
    return out
